# revision 3
# baseline (speedup 1.0000x reference)
import numpy as np

# nn_GCN_15333033247254 — hardcoded problem shapes
N = 100000
P = 8
F_IN, H, C = 128, 128, 8

_state = {}


def _bass_path(x, edge_index, W1, b1, W2, b2):
    from gcn_bass import Cfg, host_prep, build_nc, FastRunner
    cfg = Cfg()
    inputs = dict(x=x, W1=W1, b1=b1, W2=W2, b2=b2)
    if "runner" not in _state:
        prep = host_prep(cfg, edge_index)
        nc = build_nc(cfg, prep["P_tc"], prep["E_pad"])
        _state["runner"] = FastRunner(cfg, nc)
        _state["E_pad"] = prep["E_pad"]
        return _state["runner"].run(inputs, lambda: prep)

    def prep_fn():
        p = host_prep(cfg, edge_index)
        if p["E_pad"] != _state["E_pad"]:
            raise RuntimeError("layout changed; rebuild")
        return p

    try:
        return _state["runner"].run(inputs, prep_fn)
    except RuntimeError:
        _state.clear()
        return _bass_path(x, edge_index, W1, b1, W2, b2)


def _cpu_fallback(x, edge_index, W1, b1, W2, b2):
    loop = np.arange(N, dtype=np.int64)
    src = np.concatenate([np.asarray(edge_index[0]), loop])
    dst = np.concatenate([np.asarray(edge_index[1]), loop])
    deg = np.bincount(dst, minlength=N).astype(np.float32)
    dis = np.where(deg > 0, 1.0 / np.sqrt(np.maximum(deg, 1.0)), 0.0)
    norm = (dis[src] * dis[dst]).astype(np.float32)
    xw = x @ np.asarray(W1)
    msgs = xw[src] * norm[:, None]
    h = np.zeros((N, H), dtype=np.float32)
    np.add.at(h, dst, msgs)
    h = np.maximum(h + np.asarray(b1), 0.0)
    hw = h @ np.asarray(W2)
    msgs2 = hw[src] * norm[:, None]
    o = np.zeros((N, C), dtype=np.float32)
    np.add.at(o, dst, msgs2)
    o = o + np.asarray(b2)
    m = o.max(axis=1, keepdims=True)
    lse = np.log(np.exp(o - m).sum(axis=1, keepdims=True)) + m
    return (o - lse).astype(np.float32)


def kernel(x, edge_index, W1, b1, W2, b2):
    x = np.ascontiguousarray(np.asarray(x, dtype=np.float32))
    edge_index = np.asarray(edge_index)
    try:
        return _bass_path(x, edge_index, W1, b1, W2, b2)
    except Exception:
        import traceback
        traceback.print_exc()
        return _cpu_fallback(x, edge_index, W1, b1, W2, b2)
